# revision 50
# baseline (speedup 1.0000x reference)
"""CP-adapter multi-head attention on 8 Trainium2 NeuronCores.

Hardcoded for B=4, N=2048, D=1024, H=16, hd=64, R=r=64 (fp32 in/out).

Sharding: batch x head-group.  Core c owns batch c//2 and head-group
c%2 (heads 8*(c%2) .. 8*(c%2)+7 = columns [512*(c%2), 512*(c%2)+512) of
the q/k/v projections and the matching rows of the output projection).
Each core reads ONLY its batch's activations (~12 MB bf16 vs the full
100 MB a head-only TP split would stream), computes attention for its 8
heads over its batch, and emits a partial output [2048, 1024]; the host
sums each batch's two partials and adds the bias.

Kernel design:
- The CP adapter is linear (dropout p=0), so it folds on-device into
  effective weights: W_eff = W + U @ cp @ V, cp = einsum(CP_C, CP_attn).
- X streams, qkv effective weights, q/k/v, exp(S) and the proj path are
  bf16 (1 cycle/row on the PE like f32r, half the DMA/SBUF);
  accumulation is fp32 in PSUM throughout.
- DMA ring split: SP carries only the X stream and output writes; the
  ACT HWDGE ring is left free for exp; the small adapter tensors and
  the raw W staging ride the gpsimd/Pool SWDGE ring with f32->bf16 /
  f32->f32r cast-on-DMA.
- q/k are produced transposed ([dims, tokens], W_eff stationary / X^T
  moving); v is produced transposed then PE-transposed to [keys, dims]
  with a ones column appended for the softmax denominator.
- Attention per (head-pair, q-quarter): scores computed transposed,
  ST = K^T-block stationary x Q^T moving; the two heads' K=64 STs are
  packed into the PE array concurrently via row-tiling (partition
  offsets 0/64), one PSUM stripe [128 keys, 2x512 q] per key block, one
  Exp per stripe on ScalarE (free scale=1/8, no max-subtraction needed
  at these score magnitudes), then PV accumulates [V|1] stationary over
  key blocks into O' [65, 512] whose row 64 is the softmax denominator.
  Division via reciprocal + gpsimd partition-broadcast, off the PSUM
  critical path.
- Software pipeline.  The PE instruction stream is in-order, so the
  schedule is laid out to keep it dense end to end: the k-projection
  units are emitted between the effective-weight builds (PE runs k
  matmuls while the v/q/p weight DMAs and DVE adds trickle in); each
  attention group runs a one-step ST/PV software pipeline (ST_{i+1}
  issues before PV_i so the exp stream never waits on the PE FIFO); the
  remaining v/q projections and the per-quarter output projections are
  sliced into ~1-2 us thunks and paced evenly into the exp-bound
  attention stream.
"""

from contextlib import ExitStack

import numpy as np
import ml_dtypes

try:
    import concourse.bass as bass
except ImportError:  # fallback when sitecustomize paths are absent
    import sys
    sys.path.append("/opt/trn_rl_repo")
    import concourse.bass as bass
import concourse.mybir as mybir
from concourse import bacc, tile
from concourse.bass_utils import run_bass_kernel_spmd
from concourse.masks import make_identity

F32 = mybir.dt.float32
F32R = mybir.dt.float32r
BF16 = mybir.dt.bfloat16
AF = mybir.ActivationFunctionType

B, N, D = 4, 2048, 1024
H, HD = 16, 64
R = 64
NCORES = 8
CPB = 512              # output cols per core (8 heads)
NHP = 4                # head-pairs per core
NTB = 4                # 512-token tiles per batch
ATT_SCALE = HD ** -0.5
BF = ml_dtypes.bfloat16


def _build():
    nc = bacc.Bacc(None, target_bir_lowering=False, debug=False)

    # ---- external inputs (per-core views prepared on host) ----
    xqT = nc.dram_tensor("xqT", [NTB, 128, 4096], BF16, kind="ExternalInput")
    xkT = nc.dram_tensor("xkT", [NTB, 128, 4096], BF16, kind="ExternalInput")
    xvT = nc.dram_tensor("xvT", [NTB, 128, 4096], BF16, kind="ExternalInput")
    wq_c = nc.dram_tensor("wq_c", [D, CPB], F32, kind="ExternalInput")
    wk_c = nc.dram_tensor("wk_c", [D, CPB], F32, kind="ExternalInput")
    wv_c = nc.dram_tensor("wv_c", [D, CPB], F32, kind="ExternalInput")
    wp_c = nc.dram_tensor("wp_c", [CPB, D], F32, kind="ExternalInput")
    ut = nc.dram_tensor("ut", [R, D], F32, kind="ExternalInput")      # U^T
    utc = nc.dram_tensor("utc", [R, CPB], F32, kind="ExternalInput")  # U^T[:,rows_c]
    vfull = nc.dram_tensor("vfull", [R, D], F32, kind="ExternalInput")   # CP_V_W
    v_c = nc.dram_tensor("v_c", [R, CPB], F32, kind="ExternalInput")     # cols slice
    cpct = nc.dram_tensor("cpct", [R, R * R], F32, kind="ExternalInput")
    cpatt = nc.dram_tensor("cpatt", [R, 4], F32, kind="ExternalInput")

    out = nc.dram_tensor("out", [N, D], F32, kind="ExternalOutput")

    xT3 = {"q": xqT, "k": xkT, "v": xvT}
    w_dram = {"q": wq_c, "k": wk_c, "v": wv_c}

    with tile.TileContext(nc) as tc:
        with ExitStack() as es:
            const = es.enter_context(tc.tile_pool(name="const", bufs=1))
            weffp = es.enter_context(tc.tile_pool(name="weff", bufs=1))
            xstream = es.enter_context(tc.tile_pool(name="xstream", bufs=5))
            vtp = es.enter_context(tc.tile_pool(name="vtp", bufs=2))
            qkvp = es.enter_context(tc.tile_pool(name="qkv", bufs=1))
            xap = es.enter_context(tc.tile_pool(name="xap", bufs=1))
            # PSUM bank layout: ps_qkv/ps_st first so the transient prep pool
            # lands on the banks ps_o reuses later (first PV use comes long
            # after prep retires; sharing with ps_qkv would stall the lead-in
            # qkv matmuls behind the last prep matmul).
            ps_qkv = es.enter_context(
                tc.tile_pool(name="ps_qkv", bufs=2, space="PSUM"))
            ps_st = es.enter_context(
                tc.tile_pool(name="ps_st", bufs=2, space="PSUM"))
            prep_es = ExitStack()
            prep = prep_es.enter_context(tc.tile_pool(name="prep", bufs=1))
            wstage = prep_es.enter_context(tc.tile_pool(name="wstage", bufs=4))
            ps_prep = prep_es.enter_context(
                tc.tile_pool(name="ps_prep", bufs=2, space="PSUM"))
            prep0_es = ExitStack()
            prep0 = prep0_es.enter_context(tc.tile_pool(name="prep0", bufs=1))

            # cpct/cpatt ride the fast SP HWDGE ring as raw f32 (the cpT
            # minis are only 4 columns wide, so the f32 matmul-rate penalty
            # is irrelevant and skipping the SWDGE cast saves ~3us at t=0)
            cpct_yx = prep0.tile([R, R, R], F32)     # [r, y, x]
            nc.sync.dma_start(cpct_yx[:],
                              cpct.rearrange("r (y x) -> r y x", x=R))
            cpatt_r = prep.tile([R, 4], F32)
            nc.sync.dma_start(cpatt_r[:], cpatt[:])

            # ---------- prefetch lead-in X tiles (SP ring: X + out only) ---
            xs_pref = {}
            for tb in range(NTB):
                xs = xstream.tile([128, 4096], BF16, name="xs", tag="xs")
                nc.sync.dma_start(xs[:], xkT[tb])
                xs_pref[("k", tb)] = xs

            # ---------- small tensors: cast-on-DMA via the Pool SWDGE ring -
            vc_sb = prep.tile([R, CPB], F32R)
            nc.gpsimd.dma_start(vc_sb[:], v_c[:])
            ut_sb = prep.tile([R, 8, 128], F32R)
            nc.gpsimd.dma_start(ut_sb[:],
                                ut.rearrange("r (c x) -> r c x", x=128))

            def wst_halves(t):
                """Stage W[t] as two bf16 half-tensors on the Pool ring."""
                view = w_dram[t].rearrange("(ko ki) j -> ki ko j", ki=128)
                halves = []
                for h in range(2):
                    w = wstage.tile([128, 4, CPB], BF16, name="wst", tag="wst")
                    nc.gpsimd.dma_start(w[:], view[:, h * 4:(h + 1) * 4, :])
                    halves.append(w)
                return halves

            # ---------- constants ----------
            identf = const.tile([128, 128], F32)
            make_identity(nc, identf)
            identb = const.tile([128, 128], BF16)
            nc.vector.tensor_copy(identb[:], identf[:])
            onesf = const.tile([128, 1], F32)
            nc.any.memset(onesf[:], 1.0)
            # preload the exp table set off the critical path (ACT is free)
            warm = const.tile([1, 1], F32)
            nc.scalar.activation(warm[:], onesf[0:1, :], AF.Exp)

            # persistent per-core state
            qT = qkvp.tile([128, NHP, NTB, 512], BF16)   # [pair-dim, hp, tb, tok]
            kT = qkvp.tile([128, NHP, NTB, 512], BF16)
            v_b = qkvp.tile([128, 16, 8, 65], BF16)      # [key, kb, head, hd|1]
            xaT = xap.tile([128, NHP * 4, 512], BF16)    # [pair-dim, hp*4+qq, tok]

            def ones_unit():
                nc.vector.tensor_copy(
                    v_b[:, :, :, 64:65],
                    onesf[:, None, None, :].broadcast_to([128, 16, 8, 1]))

            def x_load(t, tb):
                if (t, tb) in xs_pref:
                    return xs_pref.pop((t, tb))
                xs = xstream.tile([128, 4096], BF16, name="xs", tag="xs")
                nc.sync.dma_start(xs[:], xT3[t][tb])
                return xs

            def qkv_cc(t, tb, cc, xs, chunk=None):
                """Matmuls + copy-out for one (tensor, token-tile, col-chunk).

                chunk=(ps, lo, hi) runs only db in [lo, hi) accumulating into
                ps; the hi==8 call finishes with the copy-out."""
                if chunk is None:
                    ps = ps_qkv.tile([128, 512], F32, name="psqkv",
                                     tag="psqkv")
                    lo, hi = 0, 8
                else:
                    ps, lo, hi = chunk
                for db in range(lo, hi):
                    nc.tensor.matmul(
                        ps[:], weff[t][:, db, cc * 128:(cc + 1) * 128],
                        xs[:, db * 512:(db + 1) * 512],
                        start=(db == 0), stop=(db == 7))
                if hi < 8:
                    return ps
                if t == "v":
                    vt = vtp.tile([128, 512], BF16, name="vt", tag="vt")
                    nc.vector.tensor_copy(vt[:], ps[:])
                    for j in range(4):
                        kb = tb * 4 + j
                        tp = ps_qkv.tile([128, 128], BF16, name="pstr",
                                         tag="psqkv")
                        nc.tensor.transpose(
                            tp[:], vt[:, j * 128:(j + 1) * 128], identb[:])
                        nc.vector.tensor_copy(v_b[:, kb, 2 * cc, 0:64],
                                              tp[:, 0:64])
                        nc.vector.tensor_copy(v_b[:, kb, 2 * cc + 1, 0:64],
                                              tp[:, 64:128])
                else:
                    dst = qT if t == "q" else kT
                    nc.vector.tensor_copy(dst[:, cc, tb, :], ps[:])
                return None

            def qkv_unit(t, tb):
                xs = x_load(t, tb)
                for cc in range(4):
                    qkv_cc(t, tb, cc, xs)

            ones_unit()

            # ---------- prep: cpT[y, x, f] = sum_r cpatt[r, f] cpct[r, y, x]
            # (x-paired tiny matmuls -- keeps partition dim = y and avoids a
            # DRAM-roundtrip transpose)
            cpT = prep.tile([R, R, 4], F32R)
            for x in range(0, R, 2):
                cps = ps_prep.tile([R, 8], F32, name="cps", tag="cps")
                nc.tensor.matmul(cps[:, 0:4], cpct_yx[:, :, x], cpatt_r[:],
                                 start=True, stop=True)
                nc.tensor.matmul(cps[:, 4:8], cpct_yx[:, :, x + 1],
                                 cpatt_r[:], start=True, stop=True)
                nc.vector.tensor_copy(cpT[:, x:x + 2, :], cps[:])
            prep0_es.close()

            # ---------- prep: effective qkv weights (bf16), k first, with
            # the k lead-in units emitted between the builds so the PE stays
            # dense while the v/q/p weight stages arrive.
            weff = {}

            def weff_build(t, fi):
                halves = wst_halves(t)
                t1ps = ps_prep.tile([R, CPB], F32, name="t1ps", tag="cps")
                nc.tensor.matmul(t1ps[:], cpT[:, :, fi], vc_sb[:],
                                 start=True, stop=True)
                t1 = prep.tile([R, CPB], F32R, name="t1", tag="t1")
                nc.vector.tensor_copy(t1[:], t1ps[:])
                we = weffp.tile([128, 8, CPB], BF16, name=f"weff{t}")
                weff[t] = we
                for db in range(8):
                    t2ps = ps_prep.tile([128, CPB], F32, name="t2ps",
                                        tag="cps")
                    nc.tensor.matmul(t2ps[:], ut_sb[:, db, :], t1[:],
                                     start=True, stop=True)
                    nc.vector.tensor_add(we[:, db, :],
                                         halves[db // 4][:, db % 4, :],
                                         t2ps[:])

            # Only K0 and Q0 run in the lead-in (q weights build before v so
            # the Pool-ring stage rotation doesn't serialize weff_q behind
            # the v-adds); K1-3 and the v0 slice go through the fill stream
            # of the first attention group, pulling the first exp ~35 us
            # earlier.  v0's X load is issued here so it lands before its
            # fill runs.
            weff_build("k", 1)
            qkv_unit("k", 0)
            weff_build("q", 0)
            qkv_unit("q", 0)
            weff_build("v", 2)
            v_xs = {0: x_load("v", 0)}

            # ---------- prep: effective proj weight (bf16) ----------
            vfull_sb = prep.tile([R, 2, 512], F32R)
            nc.gpsimd.dma_start(vfull_sb[:],
                                vfull.rearrange("r (c x) -> r c x", x=512))
            utc_sb = prep.tile([R, CPB], F32R)
            nc.gpsimd.dma_start(utc_sb[:], utc[:])
            t1p = prep.tile([R, 2, 512], F32R)
            for ch in range(2):
                tps = ps_prep.tile([R, 512], F32, name="tps", tag="cps")
                nc.tensor.matmul(tps[:], cpT[:, :, 3], vfull_sb[:, ch, :],
                                 start=True, stop=True)
                nc.vector.tensor_copy(t1p[:, ch, :], tps[:])
            # wp_c [CPB, D] -> [ki, rc, ch, 512]: row = rc*128+ki, col =
            # ch*512+x; staged in two rc-halves on the Pool ring.
            wp_view = wp_c.rearrange("(rc ki) (ch x) -> ki rc ch x",
                                     ki=128, x=512)
            weff_p = weffp.tile([128, 4, 2, 512], BF16)
            for h in range(2):
                wph = wstage.tile([128, 2, 2, 512], BF16, name="wst",
                                  tag="wst")
                nc.gpsimd.dma_start(wph[:], wp_view[:, h * 2:(h + 1) * 2])
                for rc in range(h * 2, h * 2 + 2):
                    for ch in range(2):
                        tps = ps_prep.tile([128, 512], F32, name="t2pps",
                                           tag="cps")
                        nc.tensor.matmul(tps[:],
                                         utc_sb[:, rc * 128:(rc + 1) * 128],
                                         t1p[:, ch, :], start=True, stop=True)
                        nc.vector.tensor_add(weff_p[:, rc, ch, :],
                                             wph[:, rc - h * 2, ch, :],
                                             tps[:])

            prep_es.close()
            ptp = es.enter_context(tc.tile_pool(name="pt", bufs=3))
            normp = es.enter_context(tc.tile_pool(name="norm", bufs=2))
            outst = es.enter_context(tc.tile_pool(name="outst", bufs=3))
            ps_o = es.enter_context(
                tc.tile_pool(name="ps_o", bufs=2, space="PSUM"))

            def make_group(hp, qq):
                o_ps = [
                    ps_o.tile([65, 512], F32, name="o_ps", tag="o_ps")
                    for _ in range(2)
                ]
                pts = {}

                def st_unit(kb):
                    def f():
                        st = ps_st.tile([128, 1024], F32, name="st", tag="st")
                        ktb, ksub = kb // 4, kb % 4
                        for hh in range(2):
                            ro = hh * 64
                            nc.tensor.matmul(
                                st[:, hh * 512:(hh + 1) * 512],
                                kT[ro:ro + 64, hp, ktb,
                                   ksub * 128:(ksub + 1) * 128],
                                qT[ro:ro + 64, hp, qq, :],
                                start=True, stop=True)
                        pt = ptp.tile([128, 1024], BF16, name="pt", tag="pt")
                        nc.scalar.activation(pt[:], st[:], AF.Exp,
                                             scale=ATT_SCALE)
                        pts[kb] = pt
                    return f

                def pv_unit(kb):
                    def f():
                        pt = pts.pop(kb)
                        for hh in range(2):
                            nc.tensor.matmul(
                                o_ps[hh][:], v_b[:, kb, 2 * hp + hh, :],
                                pt[:, hh * 512:(hh + 1) * 512],
                                start=(kb == 0), stop=(kb == 15))
                    return f

                def norm_unit():
                    for hh in range(2):
                        ro = hh * 64
                        o_sb = normp.tile([65, 512], F32, name="o_sb",
                                          tag="o_sb")
                        nc.vector.tensor_copy(o_sb[:], o_ps[hh][:])
                        rec = normp.tile([1, 512], F32, name="rec", tag="rec")
                        nc.vector.reciprocal(rec[:], o_sb[64:65, :])
                        rec64 = normp.tile([64, 512], F32, name="rec64",
                                           tag="rec64")
                        nc.gpsimd.partition_broadcast(rec64[:], rec[:])
                        nc.vector.tensor_mul(xaT[ro:ro + 64, hp * 4 + qq, :],
                                             o_sb[0:64, :], rec64[:])
                return ([st_unit(kb) for kb in range(16)],
                        [pv_unit(kb) for kb in range(16)], norm_unit)

            def proj_halves(qq):
                """Yield ~4-matmul thunks for the qq-quarter projection."""
                for sub in range(4):
                    state = {}

                    def half(ch, sub=sub, state=state):
                        def f():
                            if ch == 0:
                                state["ob"] = outst.tile([128, 1024], F32,
                                                         name="ob", tag="ob")
                            ps = ps_qkv.tile([128, 512], F32, name="pspj",
                                             tag="psqkv")
                            for hp in range(NHP):
                                nc.tensor.matmul(
                                    ps[:],
                                    xaT[:, hp * 4 + qq,
                                        sub * 128:(sub + 1) * 128],
                                    weff_p[:, hp, ch, :],
                                    start=(hp == 0), stop=(hp == 3))
                            ob = state["ob"]
                            nc.vector.tensor_copy(
                                ob[:, ch * 512:(ch + 1) * 512], ps[:])
                            if ch == 1:
                                tok0 = qq * 512 + sub * 128
                                nc.sync.dma_start(out[tok0:tok0 + 128, :],
                                                  ob[:])
                        return f
                    yield half(0)
                    yield half(1)

            # ---------- attention schedule ----------
            def v_load_thunk(tb):
                def f():
                    v_xs[tb] = x_load("v", tb)
                return f

            def k_unit_thunk(tb):
                def f():
                    qkv_unit("k", tb)
                return f

            def cc_halves(t, tb, cc, xs_of):
                """Two ~4-matmul thunks for one (tensor, tile, col-chunk)."""
                state = {}

                def half_a():
                    ps = ps_qkv.tile([128, 512], F32, name="psqkv",
                                     tag="psqkv")
                    state["ps"] = qkv_cc(t, tb, cc, xs_of(), (ps, 0, 4))

                def half_b():
                    qkv_cc(t, tb, cc, xs_of(), (state["ps"], 4, 8))
                return [half_a, half_b]

            def v_halves(tb, cc):
                return cc_halves("v", tb, cc, lambda tb=tb: v_xs[tb])

            def q_fills(tb):
                state = {}

                def load():
                    state["xs"] = x_load("q", tb)
                L = [load]
                for cc in range(4):
                    L += cc_halves("q", tb, cc, lambda: state["xs"])
                return L

            def phase_fill_list(qq):
                if qq == 0:
                    # Dependency-ordered (eager, one per slot): pv(kb) of
                    # group hpC needs v-tile kb//4 col-chunk C.  v0's chunks
                    # run early so its X buffer frees for the q1 load
                    # mid-phase (the buffer rotation maps q1 onto v0's slot).
                    # None entries are skip-slots placed where the deadline
                    # chain has slack, so the exp stream isn't starved by
                    # back-to-back fill bursts.
                    # Deadline rules: K-fill for tile T at index <= 4*T - 2
                    # (its STs start at iteration 4T); the second half of
                    # v-slice (tb=T, cc=C) at index <= 15*C + 4*T (pv(4T) of
                    # group C runs right after the fill consumed there).
                    # X-buffer rotation: ldvT's buffer frees at the K-fill of
                    # tile T, ldq1's at the lead-in Q0.
                    P = None

                    def v0c0_fill():
                        qkv_cc("v", 0, 0, v_xs[0])
                    L = [v0c0_fill]                          # idx 0  (pv0)
                    L += [k_unit_thunk(1)]                   # idx 1  <= 2
                    L += [v_load_thunk(1)]
                    L += v_halves(1, 0)                      # idx 3-4  <= 4
                    L += [k_unit_thunk(2)]                   # idx 5  <= 6
                    L += [v_load_thunk(2)]
                    L += v_halves(2, 0)                      # idx 7-8  <= 8
                    L += [k_unit_thunk(3)]                   # idx 9  <= 10
                    L += [v_load_thunk(3)]
                    L += v_halves(3, 0)                      # idx 11-12 <= 12
                    L += v_halves(0, 1)                      # idx 12-13 <= 15
                    qf1 = q_fills(1)
                    L += qf1[:1]                 # load q1 (reuses q0's buf)
                    L += v_halves(1, 1)                      # idx 15-16 <= 19
                    L += v_halves(2, 1)                      # idx 17-18 <= 23
                    L += v_halves(0, 2)                      # idx 19-20 <= 30
                    L += v_halves(3, 1) + [P]                # idx 21-22 <= 27
                    L += v_halves(0, 3)                      # idx 24-25 <= 45
                    L += v_halves(1, 2) + [P]                # idx 26-27 <= 34
                    L += v_halves(2, 2) + [P]                # idx 29-30 <= 38
                    L += v_halves(3, 2) + [P]                # idx 32-33 <= 42
                    for i, f in enumerate(qf1[1:]):
                        L += [f] if i % 2 == 0 else [f, P]
                    L += v_halves(1, 3) + [P]                # <= 49
                    L += v_halves(2, 3) + [P]                # <= 53
                    L += v_halves(3, 3)                      # <= 57
                    return L, True
                if qq == 1:
                    return q_fills(2) + list(proj_halves(0)), False
                if qq == 2:
                    return q_fills(3) + list(proj_halves(1)), False
                return list(proj_halves(2)), False

            for qq in range(4):
                fills, eager = phase_fill_list(qq)
                slot = 0
                consumed = 0
                for hp in range(NHP):
                    sts, pvs, norm = make_group(hp, qq)
                    sts[0]()
                    for kb in range(1, 16):
                        sts[kb]()
                        slot += 1
                        target = slot if eager else \
                            (slot * len(fills) + 59) // 60
                        if consumed < min(target, len(fills)):
                            f = fills[consumed]
                            consumed += 1
                            if f is not None:
                                f()
                        pvs[kb - 1]()
                    pvs[15]()
                    norm()
                for f in fills[consumed:]:
                    if f is not None:
                        f()
            for f in proj_halves(3):
                f()
    nc.compile()
    return nc


_NC = None


def _get_nc():
    global _NC
    if _NC is None:
        _NC = _build()
    return _NC


def _prep_in_maps(inputs):
    f32 = lambda a: np.ascontiguousarray(np.asarray(a), dtype=np.float32)
    xs = {t: f32(inputs[f"input_{t}"]) for t in ("q", "k", "v")}

    def tile_xt(xb):
        # [N, D] -> [4, 128, 4096]: tile t holds tokens [512t, 512t+512),
        # laid out [ki, ko*512 + j] with d = ko*128 + ki.
        xt = xb.T.astype(BF)                        # [D, N]
        xt = xt.reshape(8, 128, NTB, 512)           # ko ki t j
        xt = xt.transpose(2, 1, 0, 3)               # t ki ko j
        return np.ascontiguousarray(xt.reshape(NTB, 128, 4096))

    Wq, Wk, Wv = f32(inputs["Wq"]), f32(inputs["Wk"]), f32(inputs["Wv"])
    Wp = f32(inputs["Wproj"])
    U = f32(inputs["CP_U_W"])              # [D, R]
    V = f32(inputs["CP_V_W"])              # [R, D]
    CPC = f32(inputs["CP_C"])              # [a, b, r]
    CPATT = f32(inputs["CP_attention"])    # [R, 4]
    ut = np.ascontiguousarray(U.T)         # [R, D]
    cpct = np.ascontiguousarray(CPC.transpose(2, 1, 0).reshape(R, R * R))
    in_maps = []
    for c in range(NCORES):
        b, hg = c // 2, c % 2
        s = slice(hg * CPB, (hg + 1) * CPB)
        in_maps.append({
            "xqT": tile_xt(xs["q"][b]),
            "xkT": tile_xt(xs["k"][b]),
            "xvT": tile_xt(xs["v"][b]),
            "wq_c": np.ascontiguousarray(Wq[:, s]),
            "wk_c": np.ascontiguousarray(Wk[:, s]),
            "wv_c": np.ascontiguousarray(Wv[:, s]),
            "wp_c": np.ascontiguousarray(Wp[s, :]),
            "ut": ut,
            "utc": np.ascontiguousarray(ut[:, s]),
            "vfull": V,
            "v_c": np.ascontiguousarray(V[:, s]),
            "cpct": cpct,
            "cpatt": CPATT,
        })
    return in_maps


def run(inputs, trace=False, trace_cores=None):
    nc = _get_nc()
    in_maps = _prep_in_maps(inputs)
    res = run_bass_kernel_spmd(nc, in_maps, list(range(NCORES)),
                               trace=trace, trace_cores=trace_cores)
    bproj = np.asarray(inputs["bproj"], dtype=np.float32)
    full = np.empty((B, N, D), dtype=np.float32)
    for b in range(B):
        full[b] = res.results[2 * b]["out"].astype(np.float32)
        full[b] += res.results[2 * b + 1]["out"]
        full[b] += bproj[None, :]
    return full, res


def kernel(**inputs):
    out, _ = run(inputs, trace=False)
    return out


def _numpy_partial(inputs, c):
    """fp32 numpy model of core c's partial output (for sim verification)."""
    f32 = lambda a: np.asarray(a, dtype=np.float32)
    b, hg = c // 2, c % 2
    s = slice(hg * CPB, (hg + 1) * CPB)
    U, V = f32(inputs["CP_U_W"]), f32(inputs["CP_V_W"])
    CPc = np.einsum("xyr,rf->xyf", f32(inputs["CP_C"]),
                    f32(inputs["CP_attention"]))
    weffs = {}
    for t, wname, fi in (("q", "Wq", 0), ("k", "Wk", 1), ("v", "Wv", 2)):
        weffs[t] = (f32(inputs[wname]) + U @ CPc[..., fi] @ V)[:, s]
    wp_eff = (f32(inputs["Wproj"]) + U @ CPc[..., 3] @ V)[s, :]
    q = f32(inputs["input_q"][b]) @ weffs["q"]
    k = f32(inputs["input_k"][b]) @ weffs["k"]
    v = f32(inputs["input_v"][b]) @ weffs["v"]
    xa = np.empty((N, CPB), dtype=np.float32)
    for h in range(8):
        hs = slice(h * HD, (h + 1) * HD)
        sc = (q[:, hs] @ k[:, hs].T) * ATT_SCALE
        w = np.exp(sc - sc.max(axis=1, keepdims=True))
        w /= w.sum(axis=1, keepdims=True)
        xa[:, hs] = w @ v[:, hs]
    return xa @ wp_eff
